# revision 5
# baseline (speedup 1.0000x reference)
"""Trainium2 Bass kernel for nn_AdaptiveModalityEncoder.

Reference computation (per row r of input_data [B, D]):
    sel[r] = selection_mask[r, modality_idx] > 0.5
    out[r] = sel[r] ? gelu(x[r] @ W1 + b1) @ W2 + b2 : 0

Strategy (moe_routing, data-parallel across 8 cores):
  - Host computes the selected-row list, gathers + transposes the selected
    rows (routing metadata/prep), and splits them evenly across the 8
    cores; each core runs a pure dense 2-layer MLP in bf16 (fp32
    accumulate) over its T rows and writes a compact batch-major output.
    Host scatters the compact outputs into the zero-filled full output.
  - Device kernel is gather/scatter-free: only linear DMAs, which stripe
    across all 16 HW DMA queues (aggregate ~358 GB/s), so the PE starts
    within ~1 us and runs back-to-back matmuls (PE-bound regime).

Matmul layout: activations feature-major for L1 (X^T tiles [D_part, rows],
pre-transposed on host; H^T = W1^T @ X^T with W1 repacked on host so each
h-tile's 8 k-slices are one contiguous 256 KB load). L2 flips operands so
the output comes out batch-major (OUT = (HT)^T @ W2, lhsT = HT slices,
W2 natural layout) and is written straight out with plain DMAs.
"""

import sys

sys.path.insert(0, "/opt/trn_rl_repo")

import numpy as np
import ml_dtypes

# Problem constants (hardcoded per harness contract).
B, D, H, O, K = 16384, 1024, 2048, 1024, 4
NCORES = 8
P = 128
KD = D // P  # 8 k-tiles for layer 1
KH = H // P  # 16 k-tiles for layer 2

_GRAPH_CACHE = {}


def _build_graph(NG, has_b2, act="gelu"):
    """Build + compile the per-core Bass graph. NG = number of 128-row
    tiles per core (CP = NG*128 padded rows). Same graph on all 8 cores.
    has_b2=False specializes away the b2 add: L2 PSUM eviction runs on the
    scalar engine (Copy) and the output DMA fires from the scalar HWDGE
    queue, keeping the tail dependency chain on one engine."""
    import concourse.mybir as mybir
    import concourse.tile as tile
    from concourse import bacc

    f32 = mybir.dt.float32
    bf16 = mybir.dt.bfloat16
    act_fn = {
        "gelu": mybir.ActivationFunctionType.Gelu_apprx_tanh,
        "tanh": mybir.ActivationFunctionType.Tanh,  # CoreSim stand-in
    }[act]
    copy_fn = mybir.ActivationFunctionType.Copy

    CP = NG * P  # padded rows per core

    # Column chunks for L1 (PSUM bank = 512 fp32).
    chunks = []
    c0 = 0
    while c0 < CP:
        w = min(512, CP - c0)
        chunks.append((c0, w))
        c0 += w
    NC = len(chunks)

    nc = bacc.Bacc("TRN2", target_bir_lowering=False, debug=False, num_devices=NCORES)

    xt_d = nc.dram_tensor("xt", [D, CP], bf16, kind="ExternalInput")
    # W1 repacked host-side: block (h, k) at cols (h*KD + k)*P, i.e. one
    # h-tile = one contiguous [P, KD*P] load feeding a full k-chain.
    w1r_d = nc.dram_tensor("w1r", [P, KH * KD * P], bf16, kind="ExternalInput")
    w2_d = nc.dram_tensor("w2", [H, O], bf16, kind="ExternalInput")
    b1p_d = nc.dram_tensor("b1p", [P, KH], f32, kind="ExternalInput")
    b2r_d = nc.dram_tensor("b2r", [P, O], bf16, kind="ExternalInput")
    out_d = nc.dram_tensor("out", [CP, O], bf16, kind="ExternalOutput")

    with tile.TileContext(nc) as tc:
        with (
            tc.tile_pool(name="w1pool", bufs=KH) as w1pool,
            tc.tile_pool(name="w2pool", bufs=KH) as w2pool,
            tc.tile_pool(name="xtp", bufs=KD * NC) as xtp,
            tc.tile_pool(name="htp", bufs=KH) as htp,
            tc.tile_pool(name="outp", bufs=4) as outp,
            tc.tile_pool(name="const", bufs=1) as constp,
            tc.tile_pool(name="ps1", bufs=3, space="PSUM") as ps1,  # layer 1
            tc.tile_pool(name="ps2", bufs=3, space="PSUM") as ps2,  # layer 2
        ):
            # ---- DMA issue order = queue order: critical-path first ----
            b1_sb = constp.tile([P, KH], f32)
            nc.sync.dma_start(b1_sb[:], b1p_d[:])

            w1_sb = [
                w1pool.tile([P, KD * P], bf16, tag="w1", name=f"w1sb{h}")
                for h in range(KH)
            ]
            nc.sync.dma_start(w1_sb[0][:], w1r_d[:, 0 : KD * P])

            # X^T per (k, chunk) tiles: the first chunk's eight 128 KB tiles
            # are the only data the first L1 chain needs.
            xt_sb = [
                [
                    xtp.tile([P, cw], bf16, tag="xt", name=f"xtsb{ci}_{k}")
                    for k in range(KD)
                ]
                for ci, (c0, cw) in enumerate(chunks)
            ]
            for k in range(KD):
                nc.sync.dma_start(
                    xt_sb[0][k][:], xt_d[k * P : (k + 1) * P, 0 : chunks[0][1]]
                )

            for h in range(1, KH):
                nc.sync.dma_start(
                    w1_sb[h][:], w1r_d[:, h * KD * P : (h + 1) * KD * P]
                )

            for ci in range(1, NC):
                c0, cw = chunks[ci]
                for k in range(KD):
                    nc.sync.dma_start(
                        xt_sb[ci][k][:], xt_d[k * P : (k + 1) * P, c0 : c0 + cw]
                    )

            w2_sb = [
                w2pool.tile([P, O], bf16, tag="w2", name=f"w2sb{k}")
                for k in range(KH)
            ]
            for k in range(KH):
                nc.sync.dma_start(w2_sb[k][:], w2_d[k * P : (k + 1) * P, :])

            if has_b2:
                b2_sb = constp.tile([P, O], bf16)
                nc.sync.dma_start(b2_sb[:], b2r_d[:])

            ht_sb = [
                htp.tile([P, CP], bf16, tag="ht", name=f"htsb{h}")
                for h in range(KH)
            ]

            # ---- compute: per column chunk, L1 then L2 ----
            for ci, (c0, cw) in enumerate(chunks):
                # layer 1: H^T chunk = gelu(W1^T @ X^T + b1)
                for h in range(KH):
                    acc = ps1.tile([P, cw], f32, tag="l1acc", name=f"l1a{ci}_{h}")
                    for k in range(KD):
                        nc.tensor.matmul(
                            acc[:],
                            w1_sb[h][:, k * P : (k + 1) * P],
                            xt_sb[ci][k][:],
                            start=(k == 0),
                            stop=(k == KD - 1),
                        )
                    nc.scalar.activation(
                        ht_sb[h][:, c0 : c0 + cw],
                        acc[:],
                        act_fn,
                        bias=b1_sb[:, h : h + 1],
                    )

                # layer 2, batch-major: OUT rows = (HT slice)^T @ W2 + b2
                for rl in range(cw // P):
                    r0 = c0 + rl * P
                    for oc in range(2):
                        acc2 = ps2.tile(
                            [P, 512], f32, tag="l2acc", name=f"l2a{ci}_{rl}_{oc}"
                        )
                        for k in range(KH):
                            nc.tensor.matmul(
                                acc2[:],
                                ht_sb[k][:, r0 : r0 + P],
                                w2_sb[k][:, oc * 512 : (oc + 1) * 512],
                                start=(k == 0),
                                stop=(k == KH - 1),
                            )
                        ob = outp.tile(
                            [P, 512], bf16, tag="outsb", name=f"osb{ci}_{rl}_{oc}"
                        )
                        if has_b2:
                            nc.vector.tensor_add(
                                ob[:], acc2[:], b2_sb[:, oc * 512 : (oc + 1) * 512]
                            )
                        else:
                            # Evict on scalar so eviction + DMA below share
                            # one engine (no cross-engine semaphore hop).
                            nc.scalar.activation(ob[:], acc2[:], copy_fn)
                        eng = nc.scalar if not has_b2 else nc.sync
                        eng.dma_start(
                            out_d[r0 : r0 + P, oc * 512 : (oc + 1) * 512], ob[:]
                        )

    nc.compile()
    return nc


def _get_graph(NG, has_b2, act="gelu"):
    key = (NG, has_b2, act)
    if key not in _GRAPH_CACHE:
        _GRAPH_CACHE[key] = _build_graph(NG, has_b2, act)
    return _GRAPH_CACHE[key]


def prepare(input_data, selection_mask, W1, b1, W2, b2, modality_idx, act="gelu"):
    """Host-side routing/sharding prep. Returns (nc, in_maps, meta) or None
    if no rows are selected (output is all zeros)."""
    x = np.asarray(input_data, dtype=np.float32)
    mask = np.asarray(selection_mask, dtype=np.float32)
    midx = int(np.asarray(modality_idx))
    rows = np.nonzero(mask[:, midx] > 0.5)[0]
    total = len(rows)
    if total == 0:
        return None

    T = -(-total // NCORES)  # rows per core
    NG = -(-T // P)
    CP = NG * P
    has_b2 = bool(np.any(np.asarray(b2)))

    nc = _get_graph(NG, has_b2, act)

    bf = ml_dtypes.bfloat16
    x_bf = x.astype(bf)
    w1r = np.ascontiguousarray(
        np.asarray(W1, dtype=np.float32)
        .astype(bf)
        .reshape(KD, P, KH, P)
        .transpose(1, 2, 0, 3)
        .reshape(P, KH * KD * P)
    )
    w2_b = np.asarray(W2, dtype=np.float32).astype(bf)
    b1p = np.ascontiguousarray(
        np.asarray(b1, dtype=np.float32).reshape(KH, P).T
    )
    b2r = np.ascontiguousarray(
        np.broadcast_to(np.asarray(b2, dtype=np.float32).astype(bf), (P, O))
    )

    # Pad the global selected-row list to NCORES*CP; padding rows compute
    # garbage that the host scatter ignores.
    rows_pad = np.concatenate(
        [rows, np.full(NCORES * CP - total, rows[-1], dtype=rows.dtype)]
    )

    in_maps = []
    for i in range(NCORES):
        r_i = rows_pad[i * CP : (i + 1) * CP]
        xt = np.ascontiguousarray(x_bf[r_i].T)
        in_maps.append(
            {"xt": xt, "w1r": w1r, "w2": w2_b, "b1p": b1p, "b2r": b2r}
        )
    return nc, in_maps, (rows, total, CP)


def _assemble(res, meta):
    rows, total, CP = meta
    compact = np.concatenate(
        [np.asarray(res.results[i]["out"], dtype=np.float32) for i in range(NCORES)],
        axis=0,
    )[:total]
    out = np.zeros((B, O), dtype=np.float32)
    out[rows] = compact
    return out


def run_full(inputs, trace=False):
    """Shared by kernel() and test harness: returns (out, res)."""
    prep = prepare(**inputs)
    if prep is None:
        return np.zeros((B, O), dtype=np.float32), None
    nc, in_maps, meta = prep

    from concourse.bass_utils import run_bass_kernel_spmd

    res = run_bass_kernel_spmd(
        nc, in_maps, core_ids=list(range(NCORES)), trace=trace
    )
    return _assemble(res, meta), res


def kernel(input_data, selection_mask, W1, b1, W2, b2, modality_idx):
    out, _ = run_full(
        dict(
            input_data=input_data,
            selection_mask=selection_mask,
            W1=W1,
            b1=b1,
            W2=W2,
            b2=b2,
            modality_idx=modality_idx,
        )
    )
    return out


# revision 6
# speedup vs baseline: 1.0014x; 1.0014x over previous
"""Trainium2 Bass kernel for nn_AdaptiveModalityEncoder.

Reference computation (per row r of input_data [B, D]):
    sel[r] = selection_mask[r, modality_idx] > 0.5
    out[r] = sel[r] ? gelu(x[r] @ W1 + b1) @ W2 + b2 : 0

Strategy (moe_routing, data-parallel across 8 cores):
  - Host computes the selected-row list, gathers + transposes the selected
    rows (routing metadata/prep), and splits them evenly across the 8
    cores; each core runs a pure dense 2-layer MLP in bf16 (fp32
    accumulate) over its T rows and writes a compact batch-major output.
    Host scatters the compact outputs into the zero-filled full output.
  - Device kernel is gather/scatter-free: only linear DMAs, which stripe
    across all 16 HW DMA queues (aggregate ~358 GB/s), so the PE starts
    within ~1 us and runs back-to-back matmuls (PE-bound regime).

Matmul layout: activations feature-major for L1 (X^T tiles [D_part, rows],
pre-transposed on host; H^T = W1^T @ X^T with W1 repacked on host so each
h-tile's 8 k-slices are one contiguous 256 KB load). L2 flips operands so
the output comes out batch-major (OUT = (HT)^T @ W2, lhsT = HT slices,
W2 natural layout) and is written straight out with plain DMAs.
"""

import sys

sys.path.insert(0, "/opt/trn_rl_repo")

import numpy as np
import ml_dtypes

# Problem constants (hardcoded per harness contract).
B, D, H, O, K = 16384, 1024, 2048, 1024, 4
NCORES = 8
P = 128
KD = D // P  # 8 k-tiles for layer 1
KH = H // P  # 16 k-tiles for layer 2

_GRAPH_CACHE = {}


def _build_graph(NG, has_b2, act="gelu"):
    """Build + compile the per-core Bass graph. NG = number of 128-row
    tiles per core (CP = NG*128 padded rows). Same graph on all 8 cores.
    has_b2=False specializes away the b2 add: L2 PSUM eviction runs on the
    scalar engine (Copy) and the output DMA fires from the scalar HWDGE
    queue, keeping the tail dependency chain on one engine."""
    import concourse.mybir as mybir
    import concourse.tile as tile
    from concourse import bacc

    f32 = mybir.dt.float32
    bf16 = mybir.dt.bfloat16
    act_fn = {
        "gelu": mybir.ActivationFunctionType.Gelu_apprx_tanh,
        "tanh": mybir.ActivationFunctionType.Tanh,  # CoreSim stand-in
    }[act]
    copy_fn = mybir.ActivationFunctionType.Copy

    CP = NG * P  # padded rows per core

    # Column chunks for L1 (PSUM bank = 512 fp32).
    chunks = []
    c0 = 0
    while c0 < CP:
        w = min(512, CP - c0)
        chunks.append((c0, w))
        c0 += w
    NC = len(chunks)

    nc = bacc.Bacc("TRN2", target_bir_lowering=False, debug=False, num_devices=NCORES)

    xt_d = nc.dram_tensor("xt", [D, CP], bf16, kind="ExternalInput")
    # W1 repacked host-side: block (h, k) at cols (h*KD + k)*P, i.e. one
    # h-tile = one contiguous [P, KD*P] load feeding a full k-chain.
    w1r_d = nc.dram_tensor("w1r", [P, KH * KD * P], bf16, kind="ExternalInput")
    w2_d = nc.dram_tensor("w2", [H, O], bf16, kind="ExternalInput")
    b1p_d = nc.dram_tensor("b1p", [P, KH], f32, kind="ExternalInput")
    b2r_d = nc.dram_tensor("b2r", [P, O], bf16, kind="ExternalInput")
    out_d = nc.dram_tensor("out", [CP, O], bf16, kind="ExternalOutput")

    with tile.TileContext(nc) as tc:
        with (
            tc.tile_pool(name="w1pool", bufs=KH) as w1pool,
            tc.tile_pool(name="w2pool", bufs=KH) as w2pool,
            tc.tile_pool(name="xtp", bufs=KD * NC) as xtp,
            tc.tile_pool(name="htp", bufs=KH) as htp,
            tc.tile_pool(name="outp", bufs=4) as outp,
            tc.tile_pool(name="const", bufs=1) as constp,
            tc.tile_pool(name="ps1", bufs=3, space="PSUM") as ps1,  # layer 1
            tc.tile_pool(name="ps2", bufs=3, space="PSUM") as ps2,  # layer 2
            tc.tile_pool(name="wup", bufs=1, space="PSUM") as wup,
        ):
            # ---- PE warm-up: the Tensor engine ramps its p-state over
            # ~4 us of continuous execution (427 -> 216 ns per 512-col
            # matmul). Garbage matmuls on a memset tile keep the PE busy
            # through the initial DMA wait so real work starts at full
            # speed. Sized to end about when the first chunk's data lands.
            wu = constp.tile([P, 512], bf16)
            nc.vector.memset(wu[:], 0.0)
            wuacc = wup.tile([P, 512], f32)
            NWU = 34
            for i in range(NWU):
                nc.tensor.matmul(
                    wuacc[:], wu[:, 0:P], wu[:], start=True, stop=True
                )

            # ---- DMA issue order = queue order: critical-path first ----
            b1_sb = constp.tile([P, KH], f32)
            nc.sync.dma_start(b1_sb[:], b1p_d[:])

            w1_sb = [
                w1pool.tile([P, KD * P], bf16, tag="w1", name=f"w1sb{h}")
                for h in range(KH)
            ]

            # X^T per (k, chunk) tiles: the first chunk's eight 128 KB tiles
            # are the only data the first L1 chain needs.
            xt_sb = [
                [
                    xtp.tile([P, cw], bf16, tag="xt", name=f"xtsb{ci}_{k}")
                    for k in range(KD)
                ]
                for ci, (c0, cw) in enumerate(chunks)
            ]
            for k in range(KD):
                nc.sync.dma_start(
                    xt_sb[0][k][:], xt_d[k * P : (k + 1) * P, 0 : chunks[0][1]]
                )

            for h in range(KH):
                nc.sync.dma_start(
                    w1_sb[h][:], w1r_d[:, h * KD * P : (h + 1) * KD * P]
                )

            for ci in range(1, NC):
                c0, cw = chunks[ci]
                for k in range(KD):
                    nc.sync.dma_start(
                        xt_sb[ci][k][:], xt_d[k * P : (k + 1) * P, c0 : c0 + cw]
                    )

            w2_sb = [
                w2pool.tile([P, O], bf16, tag="w2", name=f"w2sb{k}")
                for k in range(KH)
            ]
            for k in range(KH):
                nc.sync.dma_start(w2_sb[k][:], w2_d[k * P : (k + 1) * P, :])

            if has_b2:
                b2_sb = constp.tile([P, O], bf16)
                nc.sync.dma_start(b2_sb[:], b2r_d[:])

            ht_sb = [
                htp.tile([P, CP], bf16, tag="ht", name=f"htsb{h}")
                for h in range(KH)
            ]

            # ---- compute: per column chunk, L1 then L2 ----
            for ci, (c0, cw) in enumerate(chunks):
                # layer 1: H^T chunk = gelu(W1^T @ X^T + b1)
                for h in range(KH):
                    acc = ps1.tile([P, cw], f32, tag="l1acc", name=f"l1a{ci}_{h}")
                    for k in range(KD):
                        nc.tensor.matmul(
                            acc[:],
                            w1_sb[h][:, k * P : (k + 1) * P],
                            xt_sb[ci][k][:],
                            start=(k == 0),
                            stop=(k == KD - 1),
                        )
                    nc.scalar.activation(
                        ht_sb[h][:, c0 : c0 + cw],
                        acc[:],
                        act_fn,
                        bias=b1_sb[:, h : h + 1],
                    )

                # layer 2, batch-major: OUT rows = (HT slice)^T @ W2 + b2
                for rl in range(cw // P):
                    r0 = c0 + rl * P
                    for oc in range(2):
                        acc2 = ps2.tile(
                            [P, 512], f32, tag="l2acc", name=f"l2a{ci}_{rl}_{oc}"
                        )
                        for k in range(KH):
                            nc.tensor.matmul(
                                acc2[:],
                                ht_sb[k][:, r0 : r0 + P],
                                w2_sb[k][:, oc * 512 : (oc + 1) * 512],
                                start=(k == 0),
                                stop=(k == KH - 1),
                            )
                        ob = outp.tile(
                            [P, 512], bf16, tag="outsb", name=f"osb{ci}_{rl}_{oc}"
                        )
                        if has_b2:
                            nc.vector.tensor_add(
                                ob[:], acc2[:], b2_sb[:, oc * 512 : (oc + 1) * 512]
                            )
                        else:
                            # Evict on scalar so eviction + DMA below share
                            # one engine (no cross-engine semaphore hop).
                            nc.scalar.activation(ob[:], acc2[:], copy_fn)
                        eng = nc.scalar if not has_b2 else nc.sync
                        eng.dma_start(
                            out_d[r0 : r0 + P, oc * 512 : (oc + 1) * 512], ob[:]
                        )

    nc.compile()
    return nc


def _get_graph(NG, has_b2, act="gelu"):
    key = (NG, has_b2, act)
    if key not in _GRAPH_CACHE:
        _GRAPH_CACHE[key] = _build_graph(NG, has_b2, act)
    return _GRAPH_CACHE[key]


def prepare(input_data, selection_mask, W1, b1, W2, b2, modality_idx, act="gelu"):
    """Host-side routing/sharding prep. Returns (nc, in_maps, meta) or None
    if no rows are selected (output is all zeros)."""
    x = np.asarray(input_data, dtype=np.float32)
    mask = np.asarray(selection_mask, dtype=np.float32)
    midx = int(np.asarray(modality_idx))
    rows = np.nonzero(mask[:, midx] > 0.5)[0]
    total = len(rows)
    if total == 0:
        return None

    T = -(-total // NCORES)  # rows per core
    NG = -(-T // P)
    CP = NG * P
    has_b2 = bool(np.any(np.asarray(b2)))

    nc = _get_graph(NG, has_b2, act)

    bf = ml_dtypes.bfloat16
    x_bf = x.astype(bf)
    w1r = np.ascontiguousarray(
        np.asarray(W1, dtype=np.float32)
        .astype(bf)
        .reshape(KD, P, KH, P)
        .transpose(1, 2, 0, 3)
        .reshape(P, KH * KD * P)
    )
    w2_b = np.asarray(W2, dtype=np.float32).astype(bf)
    b1p = np.ascontiguousarray(
        np.asarray(b1, dtype=np.float32).reshape(KH, P).T
    )
    b2r = np.ascontiguousarray(
        np.broadcast_to(np.asarray(b2, dtype=np.float32).astype(bf), (P, O))
    )

    # Pad the global selected-row list to NCORES*CP; padding rows compute
    # garbage that the host scatter ignores.
    rows_pad = np.concatenate(
        [rows, np.full(NCORES * CP - total, rows[-1], dtype=rows.dtype)]
    )

    in_maps = []
    for i in range(NCORES):
        r_i = rows_pad[i * CP : (i + 1) * CP]
        xt = np.ascontiguousarray(x_bf[r_i].T)
        in_maps.append(
            {"xt": xt, "w1r": w1r, "w2": w2_b, "b1p": b1p, "b2r": b2r}
        )
    return nc, in_maps, (rows, total, CP)


def _assemble(res, meta):
    rows, total, CP = meta
    compact = np.concatenate(
        [np.asarray(res.results[i]["out"], dtype=np.float32) for i in range(NCORES)],
        axis=0,
    )[:total]
    out = np.zeros((B, O), dtype=np.float32)
    out[rows] = compact
    return out


def run_full(inputs, trace=False):
    """Shared by kernel() and test harness: returns (out, res)."""
    prep = prepare(**inputs)
    if prep is None:
        return np.zeros((B, O), dtype=np.float32), None
    nc, in_maps, meta = prep

    from concourse.bass_utils import run_bass_kernel_spmd

    res = run_bass_kernel_spmd(
        nc, in_maps, core_ids=list(range(NCORES)), trace=trace
    )
    return _assemble(res, meta), res


def kernel(input_data, selection_mask, W1, b1, W2, b2, modality_idx):
    out, _ = run_full(
        dict(
            input_data=input_data,
            selection_mask=selection_mask,
            W1=W1,
            b1=b1,
            W2=W2,
            b2=b2,
            modality_idx=modality_idx,
        )
    )
    return out
